# revision 8
# baseline (speedup 1.0000x reference)
"""Trainium2 Bass kernel for nn_EncoderMemNN (MemNN encoder) — v2.

Math (see reference.py): story (M=256, B=16, S=64) token ids; C (4, V, 128)
embedding tables. Per hop h: m_A = sum_S C[h][s], prob = softmax_M(m_A @ u),
m_C = sum_S C[h+1][s], u += prob @ m_C. u starts at 0, so hop-0's softmax is
uniform: C[0] is never needed and u after hop 0 is mean_M(E1).

v2 strategy vs v1: data-parallel over batch (2 rows/core, 8 cores). The host
COMPACTS the vocab per core (np.unique over the core's 32768 tokens, ~24k
unique <= 32768) so token indices always fit int16 in a single dma_gather
call: no low/high vocab split, no filler slots, and a per-core fused table
tab[32768, 384] fp16 (= [C1|C2|C3] rows of the core's unique tokens). Gathers
are chunked and round-robined over 4 SWDGE queues (per-queue desc-gen is
~8ns/token and queues gen concurrently) with a deep tile ring so gen overlaps
DMA drain. Sentence sums split across engines: even groups accumulate on the
PE (identity-matmul chain into PSUM), odd groups on the DVE (pairwise fp16
tree). The attention tail uses a [m,b] logits layout (per-group 1-col
matmuls, no masks); each E block carries ones-columns so the softmax
denominator falls out of the o-matmul; softmax skips max-subtraction
(logits are O(1) for this model's 0.1-scale tables).
"""

import os
import numpy as np

HOPS = 3
V = 50257
D = 128
M = 256
B = 16
S = 64
NCORES = 8
BL = B // NCORES            # batch rows per core (2)
NS = BL * M                 # sentences per core (512)
P = 128
NG = NS // P                # sentence groups of 128 (4)
GB = NG // BL               # groups per batch row (2)
DCAT = HOPS * D             # 384 = fused row [C1|C2|C3]
GW = DCAT + 2               # E_all group block: [E1|E2|1|E3|1]
UP = 32768                  # compacted table rows (>= any possible unique count)

# chunk sizes per group (slots; pow2 for the DVE tree): small first chunk
# fills the drain pipeline early, small last chunk shortens the tail
SIZES = [int(x) for x in os.environ.get("K_SIZES", "8,16,16,16,8").split(",")]
assert sum(SIZES) == S
CO = [sum(SIZES[:c]) for c in range(len(SIZES))]    # slot offset of chunk c
NCHUNK = len(SIZES)
NQ = int(os.environ.get("K_NQ", "4"))               # SWDGE queues; queue = g % NQ
GB16 = int(os.environ.get("K_GB16", "7"))           # 16-slot gather ring depth
GB8 = int(os.environ.get("K_GB8", "4"))             # 8-slot gather ring depth
SUM_MODE = os.environ.get("K_SUM", "split")         # split | dve_tree | pe
SP = os.environ.get("K_SP", "0") == "1"             # dma_gather single_packet

_CACHE = {}


def _consts():
    ident = np.eye(P, dtype=np.float32)
    identg = np.eye(P, dtype=np.float16)
    i2 = np.eye(BL, dtype=np.float32)
    gsel = np.zeros((P, NG * BL), np.float32)
    for g in range(NG):
        gsel[:, g * BL + g // GB] = 1.0
    return {"ident": ident, "identg": identg, "i2": i2, "gsel": gsel}


def build(do_compile=True):
    from concourse import bacc, mybir, tile

    f32 = mybir.dt.float32
    f16 = mybir.dt.float16
    i16 = mybir.dt.int16
    Alu = mybir.AluOpType
    Act = mybir.ActivationFunctionType

    nc = bacc.Bacc(num_swdge_queues=NQ)
    tab_d = nc.declare_dram_parameter("tab", [UP, DCAT], f16, isOutput=False)
    idx_d = {}
    for g in range(NG):
        for c in range(NCHUNK):
            idx_d[g, c] = nc.declare_dram_parameter(
                f"idx{g}_{c}", [P, P * SIZES[c] // 16], i16, isOutput=False)
    ident_d = nc.declare_dram_parameter("ident", [P, P], f32, isOutput=False)
    identg_d = nc.declare_dram_parameter("identg", [P, P], f16, isOutput=False)
    i2_d = nc.declare_dram_parameter("i2", [BL, BL], f32, isOutput=False)
    gsel_d = nc.declare_dram_parameter("gsel", [P, NG * BL], f32, isOutput=False)
    out_d = nc.declare_dram_parameter("out", [BL, D], f32, isOutput=True)

    with tile.TileContext(nc) as tc:
        with (
            tc.tile_pool(name="const", bufs=1) as cpool,
            tc.tile_pool(name="gather", bufs=1) as gpool,
            tc.tile_pool(name="tree", bufs=1) as tpool,
            tc.tile_pool(name="work", bufs=2) as wpool,
            tc.tile_pool(name="ps_t", bufs=2, space="PSUM") as ps_t,
            tc.tile_pool(name="ps_e", bufs=1, space="PSUM") as ps_e,
            tc.tile_pool(name="ps_mm", bufs=1, space="PSUM") as ps_mm,
        ):
            # load indices in gather-issue order (c-major) so the first
            # wave (every queue's small chunk) starts ASAP
            idx_sb = {}
            for c in range(NCHUNK):
                for g in range(NG):
                    t = cpool.tile(list(idx_d[g, c].shape), i16, tag=f"idx{g}_{c}")
                    nc.sync.dma_start(out=t[:], in_=idx_d[g, c][:])
                    idx_sb[g, c] = t
            ident = cpool.tile([P, P], f32)
            nc.sync.dma_start(out=ident[:], in_=ident_d[:])
            identg = cpool.tile([P, P], f16)
            nc.sync.dma_start(out=identg[:], in_=identg_d[:])
            i2 = cpool.tile([BL, BL], f32)
            nc.sync.dma_start(out=i2[:], in_=i2_d[:])
            gsel = cpool.tile([P, NG * BL], f32)
            nc.sync.dma_start(out=gsel[:], in_=gsel_d[:])

            # ---- gather + sentence-sum into E_all group blocks [E1|E2|1|E3|1]
            # (the ones-columns make the o-matmul also produce the softmax
            # denominator); memset the ones-columns up front
            E_all = cpool.tile([P, NG * GW], f32)
            nc.vector.memset(E_all[:, 2 * D::GW], 1.0)
            nc.vector.memset(E_all[:, 3 * D + 1::GW], 1.0)
            F1 = cpool.tile([P, NS], f32)       # F1[:, g*P+m] = E1[g*P+m, :].T
            F2 = cpool.tile([P, NS], f32)
            # PSUM is bank-granular: pack small matmul outputs into two tiles;
            # mm2 holds usum (cols 0:D) and o|den (cols D:2D+1)
            mm2 = ps_mm.tile([BL, 2 * D + 1], f32, tag="mm2")
            us = mm2[:, 0:D]
            def use_pe(g):
                return SUM_MODE == "pe" or (SUM_MODE == "split" and g % 2 == 0)

            # issue all gathers c-major: queue = group, so each queue's first
            # gen is its group's small chunk and drains begin early
            gts = {}
            for c in range(NCHUNK):
                for g in range(NG):
                    sz = SIZES[c]
                    gt = gpool.tile([P, sz, DCAT], f16, tag=f"gt{sz}",
                                    bufs=GB8 if sz == 8 else GB16)
                    nc.gpsimd.dma_gather(
                        out_ap=gt[:], in_ap=tab_d[:], idxs_ap=idx_sb[g, c][:],
                        num_idxs=P * sz, num_idxs_reg=P * sz,
                        elem_size=DCAT, single_packet=SP,
                        queue_num=g % NQ,
                    )
                    gts[g, c] = gt

            # consume in arrival order (c-major): even groups accumulate on
            # the PE (identity-matmul chain into PSUM), odd groups on the DVE
            # (pairwise fp16 tree + running pair)
            eps = {}
            for g in range(NG):
                if use_pe(g):
                    ep = ps_e.tile([P, DCAT], f32, tag=f"eacc{g}")
                    eps[g] = ep
            lv2 = {}
            for c in range(NCHUNK):
                for g in range(NG):
                    gt = gts[g, c]
                    sz = SIZES[c]
                    if use_pe(g):
                        for r in range(sz):
                            nc.tensor.matmul(
                                out=eps[g][:], lhsT=identg[:], rhs=gt[:, r, :],
                                start=(c == 0 and r == 0),
                                stop=(c == NCHUNK - 1 and r == sz - 1),
                            )
                    else:
                        lv = gt
                        k = sz
                        while k > 2:
                            k //= 2
                            nl = tpool.tile([P, k, DCAT], f16, tag=f"tr{k}",
                                            bufs=2)
                            nc.vector.tensor_tensor(
                                out=nl[:], in0=lv[:, 0:k, :], in1=lv[:, k:2 * k, :],
                                op=Alu.add)
                            lv = nl
                        if g not in lv2:
                            l2 = tpool.tile([P, 2, DCAT], f16, tag=f"run{g}_{c % 2}")
                            nc.vector.tensor_copy(out=l2[:], in_=lv[:])
                        else:
                            l2 = tpool.tile([P, 2, DCAT], f16, tag=f"run{g}_{c % 2}")
                            nc.vector.tensor_tensor(
                                out=l2[:], in0=lv2[g][:], in1=lv[:], op=Alu.add)
                        lv2[g] = l2
                    if c < NCHUNK - 1:
                        continue
                    # group complete: final f32 writes split around the
                    # ones-columns, then this group's F transposes + us term
                    g0 = g * GW
                    if use_pe(g):
                        nc.scalar.copy(
                            out=E_all[:, g0:g0 + 2 * D], in_=eps[g][:, 0:2 * D])
                        nc.scalar.copy(
                            out=E_all[:, g0 + 2 * D + 1:g0 + 3 * D + 1],
                            in_=eps[g][:, 2 * D:3 * D])
                    else:
                        nc.vector.tensor_tensor(
                            out=E_all[:, g0:g0 + 2 * D],
                            in0=lv2[g][:, 0, 0:2 * D], in1=lv2[g][:, 1, 0:2 * D],
                            op=Alu.add)
                        nc.vector.tensor_tensor(
                            out=E_all[:, g0 + 2 * D + 1:g0 + 3 * D + 1],
                            in0=lv2[g][:, 0, 2 * D:3 * D],
                            in1=lv2[g][:, 1, 2 * D:3 * D],
                            op=Alu.add)
                    for t, F in ((0, F1), (1, F2)):
                        tp = ps_t.tile([P, P], f32, tag="tp")
                        nc.tensor.transpose(
                            out=tp[:],
                            in_=E_all[:, g0 + t * D: g0 + t * D + D],
                            identity=ident[:],
                        )
                        nc.scalar.copy(out=F[:, g * P:(g + 1) * P], in_=tp[:])
                    # hop 0: u = mean_M E1[b]; gsel routes group g to row g//GB
                    nc.tensor.matmul(
                        out=us, lhsT=gsel[:, g * BL:(g + 1) * BL],
                        rhs=E_all[:, g0: g0 + D],
                        start=(g == 0), stop=(g == NG - 1),
                    )
            u = wpool.tile([BL, D], f32, tag="u0")
            nc.scalar.activation(
                out=u[:], in_=us, func=Act.Copy, scale=1.0 / M)

            # ---- hops 1..2 (no masks: group g only scores for batch row
            # g // GB; no max-subtraction: logits are O(1) here)
            peTw = wpool.tile([P, NG * BL], f32, tag="peTw")
            nc.vector.memset(peTw[:], 0.0)
            for hop in (1, 2):
                F = F1 if hop == 1 else F2
                # mm1 packs this hop's small outputs: uc cols 0:BL, lgT BL:BL+NG
                mm1 = ps_mm.tile([P, BL + NG], f32, tag="mm1")
                uc_ps = mm1[:, 0:BL]
                nc.tensor.matmul(out=uc_ps, lhsT=u[:], rhs=i2[:],
                                 start=True, stop=True)
                uc = wpool.tile([P, BL], f32, tag="uc")
                nc.scalar.copy(out=uc[:], in_=uc_ps)
                lgT_ps = mm1[:, BL:BL + NG]
                for g in range(NG):
                    b = g // GB
                    nc.tensor.matmul(
                        out=lgT_ps[:, g:g + 1], lhsT=F[:, g * P:(g + 1) * P],
                        rhs=uc[:, b:b + 1], start=True, stop=True,
                    )
                # exp straight into the zero-interleaved peTw layout: within
                # group block g, only column b = g//GB is nonzero
                for b in range(BL):
                    nc.scalar.activation(
                        out=peTw[:, b * GB * BL + b:(b + 1) * GB * BL:BL],
                        in_=lgT_ps[:, b * GB:(b + 1) * GB],
                        func=Act.Exp, scale=1.0)
                # o|den[b, :] = sum_g peTw-col-b(g) . [E_hop | 1]
                o_ps = mm2[:, D:2 * D + 1]
                for g in range(NG):
                    g0 = g * GW
                    nc.tensor.matmul(
                        out=o_ps, lhsT=peTw[:, g * BL:(g + 1) * BL],
                        rhs=E_all[:, g0 + hop * D + (hop - 1):
                                  g0 + (hop + 1) * D + hop],
                        start=(g == 0), stop=(g == NG - 1),
                    )
                rden = wpool.tile([BL, 1], f32, tag="rden")
                nc.vector.reciprocal(out=rden[:], in_=o_ps[:, D:D + 1])
                u2 = wpool.tile([BL, D], f32, tag=f"u{hop}")
                nc.vector.scalar_tensor_tensor(
                    out=u2[:], in0=o_ps[:, 0:D], scalar=rden[:], in1=u[:],
                    op0=Alu.mult, op1=Alu.add,
                )
                u = u2

            nc.sync.dma_start(out=out_d[:], in_=u[:])
    if do_compile:
        nc.compile()
    return nc


def _wrap16(idx):
    """flat [n] int16 -> SBUF layout [128, n//16]: value i at [i%16, i//16],
    replicated to the 8 16-partition groups the Q7 cores read."""
    n = idx.shape[0]
    w = np.zeros((16, n // 16), np.int16)
    w[np.arange(n) % 16, np.arange(n) // 16] = idx
    return np.tile(w, (8, 1))


def prep_inputs(story, C):
    """Host-side: per-core compacted fused fp16 table + chunked index layouts."""
    story = np.asarray(story)
    C = np.asarray(C, dtype=np.float32)
    s = story.transpose(1, 0, 2).astype(np.int64)        # (B, M, S)
    ccat = np.concatenate([C[1], C[2], C[3]], axis=1).astype(np.float16)  # (V, 384)

    consts = _consts()
    in_maps = []
    for i in range(NCORES):
        toks = s[i * BL:(i + 1) * BL].reshape(-1)        # 32768 tokens
        uniq, inv = np.unique(toks, return_inverse=True)
        assert len(uniq) <= UP
        tab = np.zeros((UP, DCAT), np.float16)
        tab[:len(uniq)] = ccat[uniq]
        inv = inv.reshape(NS, S).astype(np.int16)        # sentence-major
        m = {"tab": tab, **consts}
        for g in range(NG):
            blk = inv[g * P:(g + 1) * P]                 # (128, 64)
            for c in range(NCHUNK):
                # slot k of chunk c, sentence p -> flat position k*128+p
                m[f"idx{g}_{c}"] = _wrap16(
                    blk[:, CO[c]:CO[c] + SIZES[c]].T.reshape(-1))
        in_maps.append(m)
    return in_maps


def run(in_maps, trace=False, **kwargs):
    from concourse.bass_utils import run_bass_kernel_spmd

    key = (tuple(SIZES), NQ, GB16, GB8, SUM_MODE, SP)
    if key not in _CACHE:
        _CACHE[key] = build()
    nc = _CACHE[key]
    res = run_bass_kernel_spmd(
        nc, in_maps, core_ids=list(range(NCORES)), trace=trace, **kwargs
    )
    out = np.concatenate([r["out"] for r in res.results], axis=0)
    return out, res


def kernel(story, C):
    in_maps = prep_inputs(story, C)
    out, _ = run(in_maps)
    return out.astype(np.float32)


# revision 9
# speedup vs baseline: 1.0855x; 1.0855x over previous
"""Trainium2 Bass kernel for nn_EncoderMemNN (MemNN encoder) — v2.

Math (see reference.py): story (M=256, B=16, S=64) token ids; C (4, V, 128)
embedding tables. Per hop h: m_A = sum_S C[h][s], prob = softmax_M(m_A @ u),
m_C = sum_S C[h+1][s], u += prob @ m_C. u starts at 0, so hop-0's softmax is
uniform: C[0] is never needed and u after hop 0 is mean_M(E1).

v2 strategy vs v1: data-parallel over batch (2 rows/core, 8 cores). The host
COMPACTS the vocab per core (np.unique over the core's 32768 tokens, ~24k
unique <= 32768) so token indices always fit int16 in a single dma_gather
call: no low/high vocab split, no filler slots, and a per-core fused table
tab[32768, 384] fp16 (= [C1|C2|C3] rows of the core's unique tokens). Gathers
are chunked and round-robined over 4 SWDGE queues (per-queue desc-gen is
~8ns/token and queues gen concurrently) with a deep tile ring so gen overlaps
DMA drain. Sentence sums split across engines: even groups accumulate on the
PE (identity-matmul chain into PSUM), odd groups on the DVE (pairwise fp16
tree). The attention tail uses a [m,b] logits layout (per-group 1-col
matmuls, no masks); each E block carries ones-columns so the softmax
denominator falls out of the o-matmul; softmax skips max-subtraction
(logits are O(1) for this model's 0.1-scale tables).
"""

import os
import numpy as np

HOPS = 3
V = 50257
D = 128
M = 256
B = 16
S = 64
NCORES = 8
BL = B // NCORES            # batch rows per core (2)
NS = BL * M                 # sentences per core (512)
P = 128
NG = NS // P                # sentence groups of 128 (4)
GB = NG // BL               # groups per batch row (2)
DCAT = HOPS * D             # 384 = fused row [C1|C2|C3]
GW = DCAT + 2               # E_all group block: [E1|E2|1|E3|1]
UP = 32768                  # compacted table rows (>= any possible unique count)

# chunk sizes per group (slots; pow2 for the DVE tree): small first chunk
# fills the drain pipeline early, small last chunk shortens the tail
SIZES = [int(x) for x in os.environ.get("K_SIZES", "8,16,16,16,8").split(",")]
assert sum(SIZES) == S
CO = [sum(SIZES[:c]) for c in range(len(SIZES))]    # slot offset of chunk c
NCHUNK = len(SIZES)
NQ = int(os.environ.get("K_NQ", "4"))               # SWDGE queues
GB16 = int(os.environ.get("K_GB16", "8"))           # 16-slot gather ring depth
GB8 = int(os.environ.get("K_GB8", "4"))             # 8-slot gather ring depth
SUM_MODE = os.environ.get("K_SUM", "split")         # split | dve_tree | pe
SP = os.environ.get("K_SP", "0") == "1"             # dma_gather single_packet

_CACHE = {}


def _consts():
    ident = np.eye(P, dtype=np.float32)
    identg = np.eye(P, dtype=np.float16)
    i2 = np.eye(BL, dtype=np.float32)
    gsel = np.zeros((P, NG * BL), np.float32)
    for g in range(NG):
        gsel[:, g * BL + g // GB] = 1.0
    return {"ident": ident, "identg": identg, "i2": i2, "gsel": gsel}


def build(do_compile=True):
    from concourse import bacc, mybir, tile

    f32 = mybir.dt.float32
    f16 = mybir.dt.float16
    i16 = mybir.dt.int16
    Alu = mybir.AluOpType
    Act = mybir.ActivationFunctionType

    nc = bacc.Bacc(num_swdge_queues=NQ)
    tab_d = nc.declare_dram_parameter("tab", [UP, DCAT], f16, isOutput=False)
    idx_d = {}
    for g in range(NG):
        for c in range(NCHUNK):
            idx_d[g, c] = nc.declare_dram_parameter(
                f"idx{g}_{c}", [P, P * SIZES[c] // 16], i16, isOutput=False)
    ident_d = nc.declare_dram_parameter("ident", [P, P], f32, isOutput=False)
    identg_d = nc.declare_dram_parameter("identg", [P, P], f16, isOutput=False)
    i2_d = nc.declare_dram_parameter("i2", [BL, BL], f32, isOutput=False)
    gsel_d = nc.declare_dram_parameter("gsel", [P, NG * BL], f32, isOutput=False)
    out_d = nc.declare_dram_parameter("out", [BL, D], f32, isOutput=True)

    with tile.TileContext(nc) as tc:
        with (
            tc.tile_pool(name="const", bufs=1) as cpool,
            tc.tile_pool(name="gather", bufs=1) as gpool,
            tc.tile_pool(name="tree", bufs=1) as tpool,
            tc.tile_pool(name="work", bufs=2) as wpool,
            tc.tile_pool(name="ps_t", bufs=2, space="PSUM") as ps_t,
            tc.tile_pool(name="ps_e", bufs=1, space="PSUM") as ps_e,
            tc.tile_pool(name="ps_mm", bufs=1, space="PSUM") as ps_mm,
        ):
            # load indices in gather-issue order (c-major) so the first
            # wave (every queue's small chunk) starts ASAP
            idx_sb = {}
            for c in range(NCHUNK):
                for g in range(NG):
                    t = cpool.tile(list(idx_d[g, c].shape), i16, tag=f"idx{g}_{c}")
                    nc.sync.dma_start(out=t[:], in_=idx_d[g, c][:])
                    idx_sb[g, c] = t
            ident = cpool.tile([P, P], f32)
            nc.sync.dma_start(out=ident[:], in_=ident_d[:])
            identg = cpool.tile([P, P], f16)
            nc.sync.dma_start(out=identg[:], in_=identg_d[:])
            i2 = cpool.tile([BL, BL], f32)
            nc.sync.dma_start(out=i2[:], in_=i2_d[:])
            gsel = cpool.tile([P, NG * BL], f32)
            nc.sync.dma_start(out=gsel[:], in_=gsel_d[:])

            # ---- gather + sentence-sum into E_all group blocks [E1|E2|1|E3|1]
            # (the ones-columns make the o-matmul also produce the softmax
            # denominator); memset the ones-columns up front
            E_all = cpool.tile([P, NG * GW], f32)
            nc.vector.memset(E_all[:, 2 * D::GW], 1.0)
            nc.vector.memset(E_all[:, 3 * D + 1::GW], 1.0)
            F1 = cpool.tile([P, NS], f32)       # F1[:, g*P+m] = E1[g*P+m, :].T
            F2 = cpool.tile([P, NS], f32)
            # PSUM is bank-granular: pack small matmul outputs into two tiles;
            # mm2 holds usum (cols 0:D) and o|den (cols D:2D+1)
            mm2 = ps_mm.tile([BL, 2 * D + 1], f32, tag="mm2")
            us = mm2[:, 0:D]
            def use_pe(g):
                return SUM_MODE == "pe" or (SUM_MODE == "split" and g % 2 == 0)

            # issue all gathers c-major with queue (g+c)%NQ: each queue's
            # first gen is a small c0 chunk (drains begin early) and every
            # group's chunks spread across all queues (groups finish together)
            gts = {}
            for c in range(NCHUNK):
                for g in range(NG):
                    sz = SIZES[c]
                    gt = gpool.tile([P, sz, DCAT], f16, tag=f"gt{sz}",
                                    bufs=GB8 if sz == 8 else GB16)
                    nc.gpsimd.dma_gather(
                        out_ap=gt[:], in_ap=tab_d[:], idxs_ap=idx_sb[g, c][:],
                        num_idxs=P * sz, num_idxs_reg=P * sz,
                        elem_size=DCAT, single_packet=SP,
                        queue_num=(g + c) % NQ,
                    )
                    gts[g, c] = gt

            # consume in arrival order (c-major): even groups accumulate on
            # the PE (identity-matmul chain into PSUM), odd groups on the DVE
            # (pairwise fp16 tree + running pair)
            eps = {}
            for g in range(NG):
                if use_pe(g):
                    ep = ps_e.tile([P, DCAT], f32, tag=f"eacc{g}")
                    eps[g] = ep
            lv2 = {}
            for c in range(NCHUNK):
                for g in range(NG):
                    gt = gts[g, c]
                    sz = SIZES[c]
                    if use_pe(g):
                        for r in range(sz):
                            nc.tensor.matmul(
                                out=eps[g][:], lhsT=identg[:], rhs=gt[:, r, :],
                                start=(c == 0 and r == 0),
                                stop=(c == NCHUNK - 1 and r == sz - 1),
                            )
                    else:
                        lv = gt
                        k = sz
                        while k > 2:
                            k //= 2
                            nl = tpool.tile([P, k, DCAT], f16, tag=f"tr{k}",
                                            bufs=2)
                            nc.vector.tensor_tensor(
                                out=nl[:], in0=lv[:, 0:k, :], in1=lv[:, k:2 * k, :],
                                op=Alu.add)
                            lv = nl
                        if g not in lv2:
                            l2 = tpool.tile([P, 2, DCAT], f16, tag=f"run{g}_{c % 2}")
                            nc.vector.tensor_copy(out=l2[:], in_=lv[:])
                        else:
                            l2 = tpool.tile([P, 2, DCAT], f16, tag=f"run{g}_{c % 2}")
                            nc.vector.tensor_tensor(
                                out=l2[:], in0=lv2[g][:], in1=lv[:], op=Alu.add)
                        lv2[g] = l2
                    if c < NCHUNK - 1:
                        continue
                    # group complete: final f32 writes split around the
                    # ones-columns, then this group's F transposes + us term
                    g0 = g * GW
                    if use_pe(g):
                        nc.scalar.copy(
                            out=E_all[:, g0:g0 + 2 * D], in_=eps[g][:, 0:2 * D])
                        nc.scalar.copy(
                            out=E_all[:, g0 + 2 * D + 1:g0 + 3 * D + 1],
                            in_=eps[g][:, 2 * D:3 * D])
                    else:
                        nc.vector.tensor_tensor(
                            out=E_all[:, g0:g0 + 2 * D],
                            in0=lv2[g][:, 0, 0:2 * D], in1=lv2[g][:, 1, 0:2 * D],
                            op=Alu.add)
                        nc.vector.tensor_tensor(
                            out=E_all[:, g0 + 2 * D + 1:g0 + 3 * D + 1],
                            in0=lv2[g][:, 0, 2 * D:3 * D],
                            in1=lv2[g][:, 1, 2 * D:3 * D],
                            op=Alu.add)
                    for t, F in ((0, F1), (1, F2)):
                        tp = ps_t.tile([P, P], f32, tag="tp")
                        nc.tensor.transpose(
                            out=tp[:],
                            in_=E_all[:, g0 + t * D: g0 + t * D + D],
                            identity=ident[:],
                        )
                        nc.scalar.copy(out=F[:, g * P:(g + 1) * P], in_=tp[:])
                    # hop 0: u = mean_M E1[b]; gsel routes group g to row g//GB
                    nc.tensor.matmul(
                        out=us, lhsT=gsel[:, g * BL:(g + 1) * BL],
                        rhs=E_all[:, g0: g0 + D],
                        start=(g == 0), stop=(g == NG - 1),
                    )
            u = wpool.tile([BL, D], f32, tag="u0")
            nc.scalar.activation(
                out=u[:], in_=us, func=Act.Copy, scale=1.0 / M)

            # ---- hops 1..2 (no masks: group g only scores for batch row
            # g // GB; no max-subtraction: logits are O(1) here)
            peTw = wpool.tile([P, NG * BL], f32, tag="peTw")
            nc.vector.memset(peTw[:], 0.0)
            for hop in (1, 2):
                F = F1 if hop == 1 else F2
                # mm1 packs this hop's small outputs: uc cols 0:BL, lgT BL:BL+NG
                mm1 = ps_mm.tile([P, BL + NG], f32, tag="mm1")
                uc_ps = mm1[:, 0:BL]
                nc.tensor.matmul(out=uc_ps, lhsT=u[:], rhs=i2[:],
                                 start=True, stop=True)
                uc = wpool.tile([P, BL], f32, tag="uc")
                nc.scalar.copy(out=uc[:], in_=uc_ps)
                lgT_ps = mm1[:, BL:BL + NG]
                for g in range(NG):
                    b = g // GB
                    nc.tensor.matmul(
                        out=lgT_ps[:, g:g + 1], lhsT=F[:, g * P:(g + 1) * P],
                        rhs=uc[:, b:b + 1], start=True, stop=True,
                    )
                # exp straight into the zero-interleaved peTw layout: within
                # group block g, only column b = g//GB is nonzero
                for b in range(BL):
                    nc.scalar.activation(
                        out=peTw[:, b * GB * BL + b:(b + 1) * GB * BL:BL],
                        in_=lgT_ps[:, b * GB:(b + 1) * GB],
                        func=Act.Exp, scale=1.0)
                # o|den[b, :] = sum_g peTw-col-b(g) . [E_hop | 1]
                o_ps = mm2[:, D:2 * D + 1]
                for g in range(NG):
                    g0 = g * GW
                    nc.tensor.matmul(
                        out=o_ps, lhsT=peTw[:, g * BL:(g + 1) * BL],
                        rhs=E_all[:, g0 + hop * D + (hop - 1):
                                  g0 + (hop + 1) * D + hop],
                        start=(g == 0), stop=(g == NG - 1),
                    )
                rden = wpool.tile([BL, 1], f32, tag="rden")
                nc.vector.reciprocal(out=rden[:], in_=o_ps[:, D:D + 1])
                u2 = wpool.tile([BL, D], f32, tag=f"u{hop}")
                nc.vector.scalar_tensor_tensor(
                    out=u2[:], in0=o_ps[:, 0:D], scalar=rden[:], in1=u[:],
                    op0=Alu.mult, op1=Alu.add,
                )
                u = u2

            nc.sync.dma_start(out=out_d[:], in_=u[:])
    if do_compile:
        nc.compile()
    return nc


def _wrap16(idx):
    """flat [n] int16 -> SBUF layout [128, n//16]: value i at [i%16, i//16],
    replicated to the 8 16-partition groups the Q7 cores read."""
    n = idx.shape[0]
    w = np.zeros((16, n // 16), np.int16)
    w[np.arange(n) % 16, np.arange(n) // 16] = idx
    return np.tile(w, (8, 1))


def prep_inputs(story, C):
    """Host-side: per-core compacted fused fp16 table + chunked index layouts."""
    story = np.asarray(story)
    C = np.asarray(C, dtype=np.float32)
    s = story.transpose(1, 0, 2).astype(np.int64)        # (B, M, S)
    ccat = np.concatenate([C[1], C[2], C[3]], axis=1).astype(np.float16)  # (V, 384)

    consts = _consts()
    in_maps = []
    for i in range(NCORES):
        toks = s[i * BL:(i + 1) * BL].reshape(-1)        # 32768 tokens
        uniq, inv = np.unique(toks, return_inverse=True)
        assert len(uniq) <= UP
        tab = np.zeros((UP, DCAT), np.float16)
        tab[:len(uniq)] = ccat[uniq]
        inv = inv.reshape(NS, S).astype(np.int16)        # sentence-major
        m = {"tab": tab, **consts}
        for g in range(NG):
            blk = inv[g * P:(g + 1) * P]                 # (128, 64)
            for c in range(NCHUNK):
                # slot k of chunk c, sentence p -> flat position k*128+p
                m[f"idx{g}_{c}"] = _wrap16(
                    blk[:, CO[c]:CO[c] + SIZES[c]].T.reshape(-1))
        in_maps.append(m)
    return in_maps


def run(in_maps, trace=False, **kwargs):
    from concourse.bass_utils import run_bass_kernel_spmd

    key = (tuple(SIZES), NQ, GB16, GB8, SUM_MODE, SP)
    if key not in _CACHE:
        _CACHE[key] = build()
    nc = _CACHE[key]
    res = run_bass_kernel_spmd(
        nc, in_maps, core_ids=list(range(NCORES)), trace=trace, **kwargs
    )
    out = np.concatenate([r["out"] for r in res.results], axis=0)
    return out, res


def kernel(story, C):
    in_maps = prep_inputs(story, C)
    out, _ = run(in_maps)
    return out.astype(np.float32)


# revision 13
# speedup vs baseline: 1.1314x; 1.0423x over previous
"""Trainium2 Bass kernel for nn_EncoderMemNN (MemNN encoder) — v2.

Math (see reference.py): story (M=256, B=16, S=64) token ids; C (4, V, 128)
embedding tables. Per hop h: m_A = sum_S C[h][s], prob = softmax_M(m_A @ u),
m_C = sum_S C[h+1][s], u += prob @ m_C. u starts at 0, so hop-0's softmax is
uniform: C[0] is never needed and u after hop 0 is mean_M(E1).

v2 strategy vs v1: data-parallel over batch (2 rows/core, 8 cores). The host
COMPACTS the vocab per core (np.unique over the core's 32768 tokens, ~24k
unique <= 32768) so token indices always fit int16 in a single dma_gather
call: no low/high vocab split, no filler slots, and a per-core fused table
tab[32768, 384] fp16 (= [C1|C2|C3] rows of the core's unique tokens). Gathers
are chunked and round-robined over 4 SWDGE queues (per-queue desc-gen is
~8ns/token and queues gen concurrently) with a deep tile ring so gen overlaps
DMA drain. Sentence sums split across engines: even groups accumulate on the
PE (identity-matmul chain into PSUM), odd groups on the DVE (pairwise fp16
tree). The attention tail uses a [m,b] logits layout (per-group 1-col
matmuls, no masks); each E block carries ones-columns so the softmax
denominator falls out of the o-matmul; softmax skips max-subtraction
(logits are O(1) for this model's 0.1-scale tables).
"""

import os
import numpy as np

HOPS = 3
V = 50257
D = 128
M = 256
B = 16
S = 64
NCORES = 8
BL = B // NCORES            # batch rows per core (2)
NS = BL * M                 # sentences per core (512)
P = 128
NG = NS // P                # sentence groups of 128 (4)
GB = NG // BL               # groups per batch row (2)
DCAT = HOPS * D             # 384 = fused row [C1|C2|C3]
GW = DCAT + 2               # E_all group block: [E1|E2|1|E3|1]
UP = 32768                  # compacted table rows (>= any possible unique count)

# chunk sizes per group (slots; pow2 for the DVE tree): small first chunk
# fills the drain pipeline early, small last chunk shortens the tail
SIZES = [int(x) for x in os.environ.get("K_SIZES", "8,16,16,16,8").split(",")]
assert sum(SIZES) == S
CO = [sum(SIZES[:c]) for c in range(len(SIZES))]    # slot offset of chunk c
NCHUNK = len(SIZES)
NQ = int(os.environ.get("K_NQ", "4"))               # SWDGE queues
GB16 = int(os.environ.get("K_GB16", "9"))           # 16-slot gather ring depth
GB8 = int(os.environ.get("K_GB8", "4"))             # 8-slot gather ring depth
SUM_MODE = os.environ.get("K_SUM", "split")         # split | dve_tree | pe
SP = os.environ.get("K_SP", "0") == "1"             # dma_gather single_packet

_CACHE = {}


def _consts():
    ident = np.eye(P, dtype=np.float32)
    identg = np.eye(P, dtype=np.float16)
    i2 = np.eye(BL, dtype=np.float32)
    gsel = np.zeros((P, NG * BL), np.float16)
    for g in range(NG):
        gsel[:, g * BL + g // GB] = 1.0
    return {"ident": ident, "identg": identg, "i2": i2, "gsel": gsel}


def build(do_compile=True):
    from concourse import bacc, mybir, tile

    f32 = mybir.dt.float32
    f16 = mybir.dt.float16
    i16 = mybir.dt.int16
    Alu = mybir.AluOpType
    Act = mybir.ActivationFunctionType

    nc = bacc.Bacc(num_swdge_queues=NQ)
    tab_d = nc.declare_dram_parameter("tab", [UP, DCAT], f16, isOutput=False)
    idx_d = {}
    for g in range(NG):
        for c in range(NCHUNK):
            idx_d[g, c] = nc.declare_dram_parameter(
                f"idx{g}_{c}", [P, P * SIZES[c] // 16], i16, isOutput=False)
    ident_d = nc.declare_dram_parameter("ident", [P, P], f32, isOutput=False)
    identg_d = nc.declare_dram_parameter("identg", [P, P], f16, isOutput=False)
    i2_d = nc.declare_dram_parameter("i2", [BL, BL], f32, isOutput=False)
    gsel_d = nc.declare_dram_parameter("gsel", [P, NG * BL], f16, isOutput=False)
    out_d = nc.declare_dram_parameter("out", [BL, D], f32, isOutput=True)

    with tile.TileContext(nc) as tc:
        with (
            tc.tile_pool(name="const", bufs=1) as cpool,
            tc.tile_pool(name="gather", bufs=1) as gpool,
            tc.tile_pool(name="tree", bufs=1) as tpool,
            tc.tile_pool(name="work", bufs=2) as wpool,
            tc.tile_pool(name="ps_t", bufs=2, space="PSUM") as ps_t,
            tc.tile_pool(name="ps_e", bufs=1, space="PSUM") as ps_e,
            tc.tile_pool(name="ps_mm", bufs=1, space="PSUM") as ps_mm,
        ):
            # load indices in gather-issue order (c-major) so the first
            # wave (every queue's small chunk) starts ASAP
            idx_sb = {}
            for c in range(NCHUNK):
                for g in range(NG):
                    t = cpool.tile(list(idx_d[g, c].shape), i16, tag=f"idx{g}_{c}")
                    nc.sync.dma_start(out=t[:], in_=idx_d[g, c][:])
                    idx_sb[g, c] = t
            ident = cpool.tile([P, P], f32)
            nc.sync.dma_start(out=ident[:], in_=ident_d[:])
            identg = cpool.tile([P, P], f16)
            nc.sync.dma_start(out=identg[:], in_=identg_d[:])
            i2 = cpool.tile([BL, BL], f32)
            nc.sync.dma_start(out=i2[:], in_=i2_d[:])
            gsel = cpool.tile([P, NG * BL], f16)
            nc.sync.dma_start(out=gsel[:], in_=gsel_d[:])

            # ---- gather + sentence-sum into E_all group blocks [E1|E2|1|E3|1]
            # (the ones-columns make the o-matmul also produce the softmax
            # denominator); memset the ones-columns up front
            E_all = cpool.tile([P, NG * GW], f16)
            nc.vector.memset(E_all[:, 2 * D::GW], 1.0)
            nc.vector.memset(E_all[:, 3 * D + 1::GW], 1.0)
            F1 = cpool.tile([P, NS], f16)       # F1[:, g*P+m] = E1[g*P+m, :].T
            F2 = cpool.tile([P, NS], f16)
            # PSUM is bank-granular: pack small matmul outputs into two tiles;
            # mm2 holds usum (cols 0:D) and o|den (cols D:2D+1)
            mm2 = ps_mm.tile([BL, 2 * D + 1], f32, tag="mm2")
            us = mm2[:, 0:D]
            def use_pe(g):
                return SUM_MODE == "pe" or (SUM_MODE == "split" and g in (0, NG - 1))

            # issue all gathers c-major with queue (g+c)%NQ: each queue's
            # first gen is a small c0 chunk (drains begin early) and every
            # group's chunks spread across all queues (groups finish together)
            gts = {}
            for c in range(NCHUNK):
                for g in range(NG):
                    sz = SIZES[c]
                    gt = gpool.tile([P, sz, DCAT], f16, tag=f"gt{sz}",
                                    bufs=GB8 if sz == 8 else GB16)
                    nc.gpsimd.dma_gather(
                        out_ap=gt[:], in_ap=tab_d[:], idxs_ap=idx_sb[g, c][:],
                        num_idxs=P * sz, num_idxs_reg=P * sz,
                        elem_size=DCAT, single_packet=SP,
                        queue_num=(g + c) % NQ,
                    )
                    gts[g, c] = gt

            # consume in arrival order (c-major): even groups accumulate on
            # the PE (identity-matmul chain into PSUM), odd groups on the DVE
            # (pairwise fp16 tree + running pair)
            eps = {}
            for g in range(NG):
                if use_pe(g):
                    ep = ps_e.tile([P, DCAT], f32, tag=f"eacc{g}")
                    eps[g] = ep
            lv2 = {}
            for c in range(NCHUNK):
                for g in range(NG):
                    gt = gts[g, c]
                    sz = SIZES[c]
                    if use_pe(g):
                        for r in range(sz):
                            nc.tensor.matmul(
                                out=eps[g][:], lhsT=identg[:], rhs=gt[:, r, :],
                                start=(c == 0 and r == 0),
                                stop=(c == NCHUNK - 1 and r == sz - 1),
                            )
                    else:
                        lv = gt
                        k = sz
                        while k > 2:
                            k //= 2
                            nl = tpool.tile([P, k, DCAT], f16, tag=f"tr{k}",
                                            bufs=2)
                            nc.vector.tensor_tensor(
                                out=nl[:], in0=lv[:, 0:k, :], in1=lv[:, k:2 * k, :],
                                op=Alu.add)
                            lv = nl
                        if g not in lv2:
                            l2 = tpool.tile([P, 2, DCAT], f16, tag=f"run{g}_{c % 2}")
                            nc.vector.tensor_copy(out=l2[:], in_=lv[:])
                        else:
                            l2 = tpool.tile([P, 2, DCAT], f16, tag=f"run{g}_{c % 2}")
                            nc.vector.tensor_tensor(
                                out=l2[:], in0=lv2[g][:], in1=lv[:], op=Alu.add)
                        lv2[g] = l2
                    if c < NCHUNK - 1:
                        continue
                    # group complete: final f32 writes split around the
                    # ones-columns, then this group's F transposes + us term
                    g0 = g * GW
                    if use_pe(g):
                        nc.scalar.copy(
                            out=E_all[:, g0:g0 + 2 * D], in_=eps[g][:, 0:2 * D])
                        nc.scalar.copy(
                            out=E_all[:, g0 + 2 * D + 1:g0 + 3 * D + 1],
                            in_=eps[g][:, 2 * D:3 * D])
                    else:
                        nc.vector.tensor_tensor(
                            out=E_all[:, g0:g0 + 2 * D],
                            in0=lv2[g][:, 0, 0:2 * D], in1=lv2[g][:, 1, 0:2 * D],
                            op=Alu.add)
                        nc.vector.tensor_tensor(
                            out=E_all[:, g0 + 2 * D + 1:g0 + 3 * D + 1],
                            in0=lv2[g][:, 0, 2 * D:3 * D],
                            in1=lv2[g][:, 1, 2 * D:3 * D],
                            op=Alu.add)
                    for t, F in ((0, F1), (1, F2)):
                        tp = ps_t.tile([P, P], f16, tag="tp")
                        nc.tensor.transpose(
                            out=tp[:],
                            in_=E_all[:, g0 + t * D: g0 + t * D + D],
                            identity=identg[:],
                        )
                        nc.scalar.copy(out=F[:, g * P:(g + 1) * P], in_=tp[:])
                    # hop 0: u = mean_M E1[b]; gsel routes group g to row g//GB
                    nc.tensor.matmul(
                        out=us, lhsT=gsel[:, g * BL:(g + 1) * BL],
                        rhs=E_all[:, g0: g0 + D],
                        start=(g == 0), stop=(g == NG - 1),
                    )
            u = wpool.tile([BL, D], f32, tag="u0")
            nc.scalar.activation(
                out=u[:], in_=us, func=Act.Copy, scale=1.0 / M)

            # ---- hops 1..2 (no masks: group g only scores for batch row
            # g // GB; no max-subtraction: logits are O(1) here)
            peTw = wpool.tile([P, NG * BL], f16, tag="peTw")
            nc.vector.memset(peTw[:], 0.0)
            for hop in (1, 2):
                F = F1 if hop == 1 else F2
                # mm1 packs this hop's small outputs: uc cols 0:BL, lgT BL:BL+NG
                mm1 = ps_mm.tile([P, BL + NG], f32, tag="mm1")
                uc_ps = mm1[:, 0:BL]
                nc.tensor.matmul(out=uc_ps, lhsT=u[:], rhs=i2[:],
                                 start=True, stop=True)
                uc = wpool.tile([P, BL], f16, tag="uc")
                nc.scalar.copy(out=uc[:], in_=uc_ps)
                lgT_ps = mm1[:, BL:BL + NG]
                for g in range(NG):
                    b = g // GB
                    nc.tensor.matmul(
                        out=lgT_ps[:, g:g + 1], lhsT=F[:, g * P:(g + 1) * P],
                        rhs=uc[:, b:b + 1], start=True, stop=True,
                    )
                # exp straight into the zero-interleaved peTw layout: within
                # group block g, only column b = g//GB is nonzero
                for b in range(BL):
                    nc.scalar.activation(
                        out=peTw[:, b * GB * BL + b:(b + 1) * GB * BL:BL],
                        in_=lgT_ps[:, b * GB:(b + 1) * GB],
                        func=Act.Exp, scale=1.0)
                # o|den[b, :] = sum_g peTw-col-b(g) . [E_hop | 1]
                o_ps = mm2[:, D:2 * D + 1]
                for g in range(NG):
                    g0 = g * GW
                    nc.tensor.matmul(
                        out=o_ps, lhsT=peTw[:, g * BL:(g + 1) * BL],
                        rhs=E_all[:, g0 + hop * D + (hop - 1):
                                  g0 + (hop + 1) * D + hop],
                        start=(g == 0), stop=(g == NG - 1),
                    )
                rden = wpool.tile([BL, 1], f32, tag="rden")
                nc.vector.reciprocal(out=rden[:], in_=o_ps[:, D:D + 1])
                u2 = wpool.tile([BL, D], f32, tag=f"u{hop}")
                nc.vector.scalar_tensor_tensor(
                    out=u2[:], in0=o_ps[:, 0:D], scalar=rden[:],
                    in1=u[:], op0=Alu.mult, op1=Alu.add,
                )
                u = u2

            nc.sync.dma_start(out=out_d[:], in_=u[:])
    if do_compile:
        nc.compile()
    return nc


def _wrap16(idx):
    """flat [n] int16 -> SBUF layout [128, n//16]: value i at [i%16, i//16],
    replicated to the 8 16-partition groups the Q7 cores read."""
    n = idx.shape[0]
    w = np.zeros((16, n // 16), np.int16)
    w[np.arange(n) % 16, np.arange(n) // 16] = idx
    return np.tile(w, (8, 1))


def prep_inputs(story, C):
    """Host-side: per-core compacted fused fp16 table + chunked index layouts."""
    story = np.asarray(story)
    C = np.asarray(C, dtype=np.float32)
    s = story.transpose(1, 0, 2).astype(np.int64)        # (B, M, S)
    ccat = np.concatenate([C[1], C[2], C[3]], axis=1).astype(np.float16)  # (V, 384)

    consts = _consts()
    in_maps = []
    for i in range(NCORES):
        toks = s[i * BL:(i + 1) * BL].reshape(-1)        # 32768 tokens
        uniq, inv = np.unique(toks, return_inverse=True)
        assert len(uniq) <= UP
        tab = np.zeros((UP, DCAT), np.float16)
        tab[:len(uniq)] = ccat[uniq]
        inv = inv.reshape(NS, S).astype(np.int16)        # sentence-major
        m = {"tab": tab, **consts}
        for g in range(NG):
            blk = inv[g * P:(g + 1) * P]                 # (128, 64)
            for c in range(NCHUNK):
                # slot k of chunk c, sentence p -> flat position k*128+p
                m[f"idx{g}_{c}"] = _wrap16(
                    blk[:, CO[c]:CO[c] + SIZES[c]].T.reshape(-1))
        in_maps.append(m)
    return in_maps


def run(in_maps, trace=False, **kwargs):
    from concourse.bass_utils import run_bass_kernel_spmd

    key = (tuple(SIZES), NQ, GB16, GB8, SUM_MODE, SP)
    if key not in _CACHE:
        _CACHE[key] = build()
    nc = _CACHE[key]
    res = run_bass_kernel_spmd(
        nc, in_maps, core_ids=list(range(NCORES)), trace=trace, **kwargs
    )
    out = np.concatenate([r["out"] for r in res.results], axis=0)
    return out, res


def kernel(story, C):
    in_maps = prep_inputs(story, C)
    out, _ = run(in_maps)
    return out.astype(np.float32)


# revision 14
# speedup vs baseline: 1.1756x; 1.0391x over previous
"""Trainium2 Bass kernel for nn_EncoderMemNN (MemNN encoder) — v2.

Math (see reference.py): story (M=256, B=16, S=64) token ids; C (4, V, 128)
embedding tables. Per hop h: m_A = sum_S C[h][s], prob = softmax_M(m_A @ u),
m_C = sum_S C[h+1][s], u += prob @ m_C. u starts at 0, so hop-0's softmax is
uniform: C[0] is never needed and u after hop 0 is mean_M(E1).

v2 strategy vs v1: data-parallel over batch (2 rows/core, 8 cores). The host
COMPACTS the vocab per core (np.unique over the core's 32768 tokens, ~24k
unique <= 32768) so token indices always fit int16 in a single dma_gather
call: no low/high vocab split, no filler slots, and a per-core fused table
tab[32768, 384] fp16 (= [C1|C2|C3] rows of the core's unique tokens). Gathers
are chunked and round-robined over 4 SWDGE queues (per-queue desc-gen is
~8ns/token and queues gen concurrently) with a deep tile ring so gen overlaps
DMA drain. Sentence sums split across engines: even groups accumulate on the
PE (identity-matmul chain into PSUM), odd groups on the DVE (pairwise fp16
tree). The attention tail uses a [m,b] logits layout (per-group 1-col
matmuls, no masks); each E block carries ones-columns so the softmax
denominator falls out of the o-matmul; softmax skips max-subtraction
(logits are O(1) for this model's 0.1-scale tables).
"""

import os
import numpy as np

HOPS = 3
V = 50257
D = 128
M = 256
B = 16
S = 64
NCORES = 8
BL = B // NCORES            # batch rows per core (2)
NS = BL * M                 # sentences per core (512)
P = 128
NG = NS // P                # sentence groups of 128 (4)
GB = NG // BL               # groups per batch row (2)
DCAT = HOPS * D             # 384 = fused row [C1|C2|C3]
GW = DCAT + 2               # E_all group block: [E1|E2|1|E3|1]
UP = 32768                  # compacted table rows (>= any possible unique count)

# chunk sizes per group (slots; pow2 for the DVE tree): small first chunk
# fills the drain pipeline early, small last chunk shortens the tail
SIZES = [int(x) for x in os.environ.get("K_SIZES", "8,16,16,16,8").split(",")]
assert sum(SIZES) == S
CO = [sum(SIZES[:c]) for c in range(len(SIZES))]    # slot offset of chunk c
NCHUNK = len(SIZES)
NQ = int(os.environ.get("K_NQ", "4"))               # SWDGE queues
GB16 = int(os.environ.get("K_GB16", "10"))           # 16-slot gather ring depth
GB8 = int(os.environ.get("K_GB8", "4"))             # 8-slot gather ring depth
SUM_MODE = os.environ.get("K_SUM", "split")         # split | dve_tree | pe
SP = os.environ.get("K_SP", "0") == "1"             # dma_gather single_packet

_CACHE = {}


def _consts():
    ident = np.eye(P, dtype=np.float32)
    identg = np.eye(P, dtype=np.float16)
    i2 = np.eye(BL, dtype=np.float32)
    gsel = np.zeros((P, NG * BL), np.float16)
    for g in range(NG):
        gsel[:, g * BL + g // GB] = 1.0
    return {"ident": ident, "identg": identg, "i2": i2, "gsel": gsel}


def build(do_compile=True):
    from concourse import bacc, mybir, tile

    f32 = mybir.dt.float32
    f16 = mybir.dt.float16
    i16 = mybir.dt.int16
    Alu = mybir.AluOpType
    Act = mybir.ActivationFunctionType

    nc = bacc.Bacc(num_swdge_queues=NQ)
    tab_d = nc.declare_dram_parameter("tab", [UP, DCAT], f16, isOutput=False)
    idx_d = {}
    for g in range(NG):
        for c in range(NCHUNK):
            idx_d[g, c] = nc.declare_dram_parameter(
                f"idx{g}_{c}", [P, P * SIZES[c] // 16], i16, isOutput=False)
    ident_d = nc.declare_dram_parameter("ident", [P, P], f32, isOutput=False)
    identg_d = nc.declare_dram_parameter("identg", [P, P], f16, isOutput=False)
    i2_d = nc.declare_dram_parameter("i2", [BL, BL], f32, isOutput=False)
    gsel_d = nc.declare_dram_parameter("gsel", [P, NG * BL], f16, isOutput=False)
    out_d = nc.declare_dram_parameter("out", [BL, D], f32, isOutput=True)

    with tile.TileContext(nc) as tc:
        with (
            tc.tile_pool(name="const", bufs=1) as cpool,
            tc.tile_pool(name="gather", bufs=1) as gpool,
            tc.tile_pool(name="tree", bufs=1) as tpool,
            tc.tile_pool(name="work", bufs=2) as wpool,
            tc.tile_pool(name="ps_t", bufs=2, space="PSUM") as ps_t,
            tc.tile_pool(name="ps_e", bufs=1, space="PSUM") as ps_e,
            tc.tile_pool(name="ps_mm", bufs=1, space="PSUM") as ps_mm,
        ):
            # load indices in gather-issue order (c-major) so the first
            # wave (every queue's small chunk) starts ASAP
            idx_sb = {}
            for c in range(NCHUNK):
                for g in range(NG):
                    t = cpool.tile(list(idx_d[g, c].shape), i16, tag=f"idx{g}_{c}")
                    nc.sync.dma_start(out=t[:], in_=idx_d[g, c][:])
                    idx_sb[g, c] = t
            ident = cpool.tile([P, P], f32)
            nc.sync.dma_start(out=ident[:], in_=ident_d[:])
            identg = cpool.tile([P, P], f16)
            nc.sync.dma_start(out=identg[:], in_=identg_d[:])
            i2 = cpool.tile([BL, BL], f32)
            nc.sync.dma_start(out=i2[:], in_=i2_d[:])
            gsel = cpool.tile([P, NG * BL], f16)
            nc.sync.dma_start(out=gsel[:], in_=gsel_d[:])

            # ---- gather + sentence-sum into E_all group blocks [E1|E2|1|E3|1]
            # (the ones-columns make the o-matmul also produce the softmax
            # denominator); memset the ones-columns up front
            E_all = cpool.tile([P, NG * GW], f16)
            nc.vector.memset(E_all[:, 2 * D::GW], 1.0)
            nc.vector.memset(E_all[:, 3 * D + 1::GW], 1.0)
            F1 = cpool.tile([P, NS], f16)       # F1[:, g*P+m] = E1[g*P+m, :].T
            F2 = cpool.tile([P, NS], f16)
            # PSUM is bank-granular: pack small matmul outputs into two tiles;
            # mm2 holds usum (cols 0:D) and o|den (cols D:2D+1)
            mm2 = ps_mm.tile([BL, 2 * D + 1], f32, tag="mm2")
            us = mm2[:, 0:D]
            def use_pe(g):
                return SUM_MODE == "pe" or (SUM_MODE == "split" and g == NG - 1)

            wupidx = cpool.tile([P, 8], i16, tag="wupidx")
            nc.vector.memset(wupidx[:], 0)
            wupout = cpool.tile([P, 1, DCAT], f16, tag="wupout")
            nc.gpsimd.dma_gather(
                out_ap=wupout[:], in_ap=tab_d[:], idxs_ap=wupidx[:],
                num_idxs=P, num_idxs_reg=P, elem_size=DCAT,
                single_packet=SP, queue_num=0,
            )
            # issue all gathers c-major with queue (g+c)%NQ: each queue's
            # first gen is a small c0 chunk (drains begin early) and every
            # group's chunks spread across all queues (groups finish together)
            gts = {}
            for c in range(NCHUNK):
                for g in range(NG):
                    sz = SIZES[c]
                    gt = gpool.tile([P, sz, DCAT], f16, tag=f"gt{sz}",
                                    bufs=GB8 if sz == 8 else GB16)
                    nc.gpsimd.dma_gather(
                        out_ap=gt[:], in_ap=tab_d[:], idxs_ap=idx_sb[g, c][:],
                        num_idxs=P * sz, num_idxs_reg=P * sz,
                        elem_size=DCAT, single_packet=SP,
                        queue_num=(g + c) % NQ,
                    )
                    gts[g, c] = gt

            # consume in arrival order (c-major): even groups accumulate on
            # the PE (identity-matmul chain into PSUM), odd groups on the DVE
            # (pairwise fp16 tree + running pair)
            eps = {}
            for g in range(NG):
                if use_pe(g):
                    ep = ps_e.tile([P, DCAT], f32, tag=f"eacc{g}")
                    eps[g] = ep
            lv2 = {}
            for c in range(NCHUNK):
                for g in range(NG):
                    gt = gts[g, c]
                    sz = SIZES[c]
                    if use_pe(g):
                        for r in range(sz):
                            nc.tensor.matmul(
                                out=eps[g][:], lhsT=identg[:], rhs=gt[:, r, :],
                                start=(c == 0 and r == 0),
                                stop=(c == NCHUNK - 1 and r == sz - 1),
                            )
                    else:
                        lv = gt
                        k = sz
                        while k > 2:
                            k //= 2
                            nl = tpool.tile([P, k, DCAT], f16, tag=f"tr{k}",
                                            bufs=2)
                            nc.vector.tensor_tensor(
                                out=nl[:], in0=lv[:, 0:k, :], in1=lv[:, k:2 * k, :],
                                op=Alu.add)
                            lv = nl
                        if g not in lv2:
                            l2 = tpool.tile([P, 2, DCAT], f16, tag=f"run{g}_{c % 2}")
                            nc.vector.tensor_copy(out=l2[:], in_=lv[:])
                        else:
                            l2 = tpool.tile([P, 2, DCAT], f16, tag=f"run{g}_{c % 2}")
                            nc.vector.tensor_tensor(
                                out=l2[:], in0=lv2[g][:], in1=lv[:], op=Alu.add)
                        lv2[g] = l2
                    if c < NCHUNK - 1:
                        continue
                    # group complete: final f32 writes split around the
                    # ones-columns, then this group's F transposes + us term
                    g0 = g * GW
                    if use_pe(g):
                        nc.scalar.copy(
                            out=E_all[:, g0:g0 + 2 * D], in_=eps[g][:, 0:2 * D])
                        nc.scalar.copy(
                            out=E_all[:, g0 + 2 * D + 1:g0 + 3 * D + 1],
                            in_=eps[g][:, 2 * D:3 * D])
                    else:
                        nc.vector.tensor_tensor(
                            out=E_all[:, g0:g0 + 2 * D],
                            in0=lv2[g][:, 0, 0:2 * D], in1=lv2[g][:, 1, 0:2 * D],
                            op=Alu.add)
                        nc.vector.tensor_tensor(
                            out=E_all[:, g0 + 2 * D + 1:g0 + 3 * D + 1],
                            in0=lv2[g][:, 0, 2 * D:3 * D],
                            in1=lv2[g][:, 1, 2 * D:3 * D],
                            op=Alu.add)
                    for t, F in ((0, F1), (1, F2)):
                        tp = ps_t.tile([P, P], f16, tag="tp")
                        nc.tensor.transpose(
                            out=tp[:],
                            in_=E_all[:, g0 + t * D: g0 + t * D + D],
                            identity=identg[:],
                        )
                        nc.scalar.copy(out=F[:, g * P:(g + 1) * P], in_=tp[:])
                    # hop 0: u = mean_M E1[b]; gsel routes group g to row g//GB
                    nc.tensor.matmul(
                        out=us, lhsT=gsel[:, g * BL:(g + 1) * BL],
                        rhs=E_all[:, g0: g0 + D],
                        start=(g == 0), stop=(g == NG - 1),
                    )
            u = wpool.tile([BL, D], f32, tag="u0")
            nc.scalar.activation(
                out=u[:], in_=us, func=Act.Copy, scale=1.0 / M)

            # ---- hops 1..2 (no masks: group g only scores for batch row
            # g // GB; no max-subtraction: logits are O(1) here)
            peTw = wpool.tile([P, NG * BL], f16, tag="peTw")
            nc.vector.memset(peTw[:], 0.0)
            for hop in (1, 2):
                F = F1 if hop == 1 else F2
                # mm1 packs this hop's small outputs: uc cols 0:BL, lgT BL:BL+NG
                mm1 = ps_mm.tile([P, BL + NG], f32, tag="mm1")
                uc_ps = mm1[:, 0:BL]
                nc.tensor.matmul(out=uc_ps, lhsT=u[:], rhs=i2[:],
                                 start=True, stop=True)
                uc = wpool.tile([P, BL], f16, tag="uc")
                nc.scalar.copy(out=uc[:], in_=uc_ps)
                lgT_ps = mm1[:, BL:BL + NG]
                for g in range(NG):
                    b = g // GB
                    nc.tensor.matmul(
                        out=lgT_ps[:, g:g + 1], lhsT=F[:, g * P:(g + 1) * P],
                        rhs=uc[:, b:b + 1], start=True, stop=True,
                    )
                # exp straight into the zero-interleaved peTw layout: within
                # group block g, only column b = g//GB is nonzero
                for b in range(BL):
                    nc.scalar.activation(
                        out=peTw[:, b * GB * BL + b:(b + 1) * GB * BL:BL],
                        in_=lgT_ps[:, b * GB:(b + 1) * GB],
                        func=Act.Exp, scale=1.0)
                # o|den[b, :] = sum_g peTw-col-b(g) . [E_hop | 1]
                o_ps = mm2[:, D:2 * D + 1]
                for g in range(NG):
                    g0 = g * GW
                    nc.tensor.matmul(
                        out=o_ps, lhsT=peTw[:, g * BL:(g + 1) * BL],
                        rhs=E_all[:, g0 + hop * D + (hop - 1):
                                  g0 + (hop + 1) * D + hop],
                        start=(g == 0), stop=(g == NG - 1),
                    )
                rden = wpool.tile([BL, 1], f32, tag="rden")
                nc.vector.reciprocal(out=rden[:], in_=o_ps[:, D:D + 1])
                u2 = wpool.tile([BL, D], f32, tag=f"u{hop}")
                nc.vector.scalar_tensor_tensor(
                    out=u2[:], in0=o_ps[:, 0:D], scalar=rden[:],
                    in1=u[:], op0=Alu.mult, op1=Alu.add,
                )
                u = u2

            nc.sync.dma_start(out=out_d[:], in_=u[:])
    if do_compile:
        nc.compile()
    return nc


def _wrap16(idx):
    """flat [n] int16 -> SBUF layout [128, n//16]: value i at [i%16, i//16],
    replicated to the 8 16-partition groups the Q7 cores read."""
    n = idx.shape[0]
    w = np.zeros((16, n // 16), np.int16)
    w[np.arange(n) % 16, np.arange(n) // 16] = idx
    return np.tile(w, (8, 1))


def prep_inputs(story, C):
    """Host-side: per-core compacted fused fp16 table + chunked index layouts."""
    story = np.asarray(story)
    C = np.asarray(C, dtype=np.float32)
    s = story.transpose(1, 0, 2).astype(np.int64)        # (B, M, S)
    ccat = np.concatenate([C[1], C[2], C[3]], axis=1).astype(np.float16)  # (V, 384)

    consts = _consts()
    in_maps = []
    for i in range(NCORES):
        toks = s[i * BL:(i + 1) * BL].reshape(-1)        # 32768 tokens
        uniq, inv = np.unique(toks, return_inverse=True)
        assert len(uniq) <= UP
        tab = np.zeros((UP, DCAT), np.float16)
        tab[:len(uniq)] = ccat[uniq]
        inv = inv.reshape(NS, S).astype(np.int16)        # sentence-major
        m = {"tab": tab, **consts}
        for g in range(NG):
            blk = inv[g * P:(g + 1) * P]                 # (128, 64)
            for c in range(NCHUNK):
                # slot k of chunk c, sentence p -> flat position k*128+p
                m[f"idx{g}_{c}"] = _wrap16(
                    blk[:, CO[c]:CO[c] + SIZES[c]].T.reshape(-1))
        in_maps.append(m)
    return in_maps


def run(in_maps, trace=False, **kwargs):
    from concourse.bass_utils import run_bass_kernel_spmd

    key = (tuple(SIZES), NQ, GB16, GB8, SUM_MODE, SP)
    if key not in _CACHE:
        _CACHE[key] = build()
    nc = _CACHE[key]
    res = run_bass_kernel_spmd(
        nc, in_maps, core_ids=list(range(NCORES)), trace=trace, **kwargs
    )
    out = np.concatenate([r["out"] for r in res.results], axis=0)
    return out, res


def kernel(story, C):
    in_maps = prep_inputs(story, C)
    out, _ = run(in_maps)
    return out.astype(np.float32)
